# revision 27
# baseline (speedup 1.0000x reference)
"""Trainium2 Bass kernel for MixActivConv2d (mixed-precision fake-quant + 1x1 conv).

Reference computation:
  sel = x[:, ch]                                   # gather 8 channels
  activ = sum_i softmax(aa)[i] * uq(sel, bit_i)    # global-minmax fake quant
  x_q = x with sel channels replaced by activ
  w_q = sum_i softmax(aw)[i] * uq(w, bit_i)
  out = conv1x1(x_q, w_q)  ==  w_q[256,256] @ x_q[b, 256, 4096]

Strategy (8 cores, data-parallel over batch, 4 batches/core):
  - fp16 matmul datapath: x streams in fp16 (half the HBM read), the
    exactly-quantized weights/activations are written as fp16, PSUM (fp32)
    evicts with an fp16 cast (half the HBM write), host upcasts.
    Quantization bucket decisions stay bit-exact fp32 (a flipped 2-bit
    bucket near a tie would cost ~0.05 rel err); the fp16 rounding after
    quantization only adds ~1e-3 smooth error vs the 2e-2 gate.
  - host permutes the channel axis so the 8 selected channels sit at
    partitions 120..127 of the k-chunk-1 rhs tile (wt rows permuted to
    match; conv is permutation-invariant in the contraction index).
  - batches 1..3: the quantized activations are DMA-scattered over the
    raw rows of the loaded rhs tile (4 rectangular SBUF->SBUF DMAs per
    batch), so their matmuls directly produce the final output.
  - batch 0 instead keeps a rank-8 PSUM correction (main matmuls on raw
    x + K=64 corr matmuls with activ-x), hiding the global min/max
    reduction latency behind batch 0's main matmuls.
  - global sel min/max from a replicated copy of the gathered channels
    (4 MB), reduced on-device on every core (no collectives); fused
    tensor_tensor_reduce halves the reduction passes.
  - rounding via the fp32 magic-number trick (round-to-nearest-even,
    matching jnp.round)
"""

import sys
from contextlib import ExitStack

import numpy as np

sys.path.insert(0, "/opt/trn_rl_repo")

import concourse.bass as bass  # noqa: E402
import concourse.mybir as mybir  # noqa: E402
import concourse.tile as tile  # noqa: E402
from concourse import bacc  # noqa: E402

NCORES = 8
B, C, H, W = 32, 256, 64, 64
HW = H * W  # 4096
BPC = B // NCORES  # batches per core = 4
NSEL = 8
QMAX = (3.0, 15.0, 255.0)  # 2^bit - 1 for bits (2, 4, 8)
MAGIC = 12582912.0  # 1.5 * 2**23: x + MAGIC - MAGIC == rne-round(x) for |x| < 2^22
F32 = mybir.dt.float32
F16 = mybir.dt.float16
ALU = mybir.AluOpType
AXIS = mybir.AxisListType
ACTF = mybir.ActivationFunctionType
SELP0 = 120  # sel channels live at partitions 120..127 of k-chunk 1


def _emit_scalar_consts(nc, vals, scal_mx, scal_mn, sw, tmp, d3, y3, eng=None):
    """Scalar chain on partition 0. Writes vals [1,10]:
    cols 0..2 inv_i (=1/scale_i), 3..5 k_i (=sw_i*scale_i), 6 mn, 7 MAGIC.

    scale_i = fp32-exact (mx-mn)/qmax_i via one Newton step with an exact
    (Dekker) residual: the divisors fit in 12 bits so their Veltkamp low
    split is zero and every product in the error term is exact. Verified
    bit-identical to IEEE fp32 division over millions of samples.
    d3/y3: [1,3] const tiles holding qmax_i and fl(1/qmax_i).
    tmp is a [1, 40] scratch tile.
    """

    eng = eng if eng is not None else nc.vector

    def col3(j):
        return tmp[0:1, j : j + 3]

    rng = tmp[0:1, 36:37]
    eng.tensor_sub(rng, scal_mx, scal_mn)
    n_b = rng.to_broadcast((1, 3))
    q0, p, ca, t1, ah, al, t2, t3, t4, e, t5, r = (col3(3 * j) for j in range(12))
    eng.tensor_mul(q0, n_b, y3)
    eng.tensor_mul(p, q0, d3)
    eng.tensor_scalar(ca, q0, 4097.0, None, op0=ALU.mult)
    eng.tensor_sub(t1, ca, q0)
    eng.tensor_sub(ah, ca, t1)
    eng.tensor_sub(al, q0, ah)
    eng.tensor_mul(t2, ah, d3)
    eng.tensor_sub(t3, t2, p)
    eng.tensor_mul(t4, al, d3)
    eng.tensor_add(e, t3, t4)
    eng.tensor_sub(t5, n_b, p)
    eng.tensor_sub(r, t5, e)
    scale3 = col3(0)  # reuse q0's slot via separate name for clarity
    eng.tensor_mul(t2, r, y3)  # t2 = r*y
    eng.tensor_add(scale3, q0, t2)  # scale3 overwrites q0 in place
    # inv_i = 1/scale_i (bit-exact reciprocal); k_i = sw_i * scale_i
    recip_inst = nc.vector.reciprocal(vals[0:1, 0:3], scale3)
    eng.tensor_mul(vals[0:1, 3:6], scale3, sw)
    eng.tensor_copy(vals[0:1, 6:7], scal_mn)
    eng.memset(vals[0:1, 7:8], MAGIC)
    return recip_inst


def _emit_quant(nc, pool, src, cbuf, nparts, nfree, out=None, eng=None, sfx="",
                u_pre=None, delta_out=None, delta_src=None, delta_rows=0):
    """Emit the 3-bit blended fake-quant of src [nparts, nfree].

    u = src - mn
    r_i = u*inv_i + MAGIC          (the fp32 add rounds to integer, RNE)
    p_i = (r_i - MAGIC) * k_i      (subtract is exact, result = round(u/scale)*k)
    out = p0 + p1 + p2 + mn        (written to `out`, any dtype)
    If delta_out given: also delta_out[0:delta_rows] =
        (acc + mn) - delta_src   on the first delta_rows partitions.
    Returns the output tile ([nparts, nfree]).
    """
    eng = eng if eng is not None else nc.vector
    if u_pre is not None:
        u = u_pre
    else:
        u = pool.tile([nparts, nfree], F32, tag=f"qu_{nparts}_{nfree}{sfx}", name="qu")
        eng.tensor_scalar(u, src, cbuf[:, 6:7], None, op0=ALU.subtract)
    p = []
    for i in range(3):
        # all on DVE/Pool, in place: per-op IEEE fp32 rounding must match the
        # reference's separate mul/add ops (ACT's fused internal arithmetic
        # flips near-tie elements into the next quant bucket on HW)
        pi = pool.tile(
            [nparts, nfree], F32, tag=f"ptmp{i}_{nparts}_{nfree}{sfx}", name=f"ptmp{i}"
        )
        eng.tensor_scalar(pi, u, cbuf[:, i : i + 1], None, op0=ALU.mult)
        eng.tensor_scalar(pi, pi, MAGIC, None, op0=ALU.add)
        eng.tensor_scalar(
            pi, pi, MAGIC, cbuf[:, 3 + i : 4 + i], op0=ALU.subtract, op1=ALU.mult
        )
        p.append(pi)
    eng.tensor_add(p[0], p[0], p[1])
    eng.tensor_add(p[0], p[0], p[2])
    outt = out if out is not None else pool.tile(
        [nparts, nfree], F32, tag=f"qout_{nparts}_{nfree}{sfx}", name="qout"
    )
    eng.tensor_scalar(outt, p[0], cbuf[:, 6:7], None, op0=ALU.add)
    if delta_out is not None:
        nc.vector.scalar_tensor_tensor(
            delta_out[0:delta_rows, :],
            p[0][0:delta_rows, :],
            cbuf[0:delta_rows, 6:7],
            delta_src[0:delta_rows, :],
            op0=ALU.add,
            op1=ALU.subtract,
        )
    return outt


def _kernel_body(ctx, tc, ch, x_ap, selred_ap, selloc_ap, w_ap, ws_ap, al_ap, out_ap):
    nc = tc.nc
    import concourse.bass_isa as bass_isa

    const = ctx.enter_context(tc.tile_pool(name="const", bufs=1))
    rhs_pool = ctx.enter_context(tc.tile_pool(name="rhs", bufs=2))
    out_pool = ctx.enter_context(tc.tile_pool(name="outsb", bufs=2))
    psB = ctx.enter_context(tc.tile_pool(name="psB", bufs=8, space="PSUM"))

    # ---- inputs ----
    alphas = const.tile([1, 6], F32)
    nc.gpsimd.dma_start(alphas[:], al_ap)
    # W arrives pre-transposed and channel-permuted from the host
    wtside = const.tile([128, 2 * C], F32)  # W^T chunks side by side
    nc.sync.dma_start(wtside[:, 0:C], w_ap[0:128, :])
    nc.sync.dma_start(wtside[:, C : 2 * C], w_ap[128:256, :])
    wseltraw = const.tile([NSEL, C], F32)
    nc.sync.dma_start(wseltraw[:], ws_ap)
    # replicated gathered-channel copy for the global min/max:
    # c2/c3 via SWDGE (async transfer, cheap launch), c0/c1 on the ACT
    # HWDGE queue which is otherwise idle until eviction time
    selredc = [
        const.tile([128, 2048], F32, name=f"selredc{i}", tag=f"selredc{i}")
        for i in range(4)
    ]
    nc.gpsimd.dma_start(selredc[2][:], selred_ap[:, 2 * 2048 : 3 * 2048])
    nc.gpsimd.dma_start(selredc[3][:], selred_ap[:, 3 * 2048 : 4 * 2048])
    nc.scalar.dma_start(selredc[0][:], selred_ap[:, 0:2048])
    nc.scalar.dma_start(selredc[1][:], selred_ap[:, 2048 : 2 * 2048])
    selloc = const.tile([128, 1024], F32)
    nc.gpsimd.dma_start(selloc[:], selloc_ap)

    with tc.high_priority():
        # ---- softmax of both alpha vectors (on partition 0) ----
        ex = const.tile([1, 6], F32)
        nc.scalar.activation(ex[:], alphas[:], ACTF.Exp)
        sums = const.tile([1, 8], F32)
        nc.vector.tensor_reduce(sums[0:1, 0:1], ex[0:1, 0:3], axis=AXIS.X, op=ALU.add)
        nc.vector.tensor_reduce(sums[0:1, 1:2], ex[0:1, 3:6], axis=AXIS.X, op=ALU.add)
        nc.vector.reciprocal(sums[0:1, 2:3], sums[0:1, 0:1])
        nc.vector.reciprocal(sums[0:1, 3:4], sums[0:1, 1:2])
        sw = const.tile([1, 6], F32)  # cols 0..2 = sw_activ, 3..5 = sw_weight
        nc.vector.tensor_scalar(sw[0:1, 0:3], ex[0:1, 0:3], sums[0:1, 2:3], None, op0=ALU.mult)
        nc.vector.tensor_scalar(
            sw[0:1, 3:6], ex[0:1, 3:6], sums[0:1, 3:4], None, op0=ALU.mult
        )

        # qmax and fl(1/qmax) constant vectors for the exact-division sequence
        d3 = const.tile([1, 3], F32)
        y3 = const.tile([1, 3], F32)
        for i, qm in enumerate(QMAX):
            nc.gpsimd.memset(d3[0:1, i : i + 1], float(qm))
            nc.gpsimd.memset(y3[0:1, i : i + 1], float(np.float32(1.0) / np.float32(qm)))

        # ---- W min/max: DVE free-axis partials + gpsimd cross-partition ----
        scal = const.tile([1, 8], F32)  # 0 smx, 1 smn, 2 wmx, 3 wmn
        wp = const.tile([128, 2], F32)
        wg = const.tile([128, 2], F32)
        nc.vector.tensor_reduce(wp[:, 0:1], wtside[:], axis=AXIS.X, op=ALU.max)
        nc.vector.tensor_reduce(
            wp[:, 1:2], wtside[:], axis=AXIS.X, op=ALU.min, negate=True
        )
        nc.gpsimd.partition_all_reduce(
            wg[:, 0:2], wp[:, 0:2], channels=128, reduce_op=bass_isa.ReduceOp.max
        )
        nc.gpsimd.tensor_copy(scal[0:1, 2:3], wg[0:1, 0:1])
        nc.gpsimd.tensor_scalar(scal[0:1, 3:4], wg[0:1, 1:2], -1.0, None, op0=ALU.mult)
        mnbw = const.tile([128, 1], F32)
        nc.gpsimd.partition_broadcast(mnbw[:], scal[0:1, 3:4])
        uw = const.tile([128, 2 * C], F32)
        nc.gpsimd.tensor_scalar(uw[:], wtside[:], mnbw[:, 0:1], None, op0=ALU.subtract)

        # ---- W consts + quantized weights (fp16 lhsT) ----
        valsw = const.tile([1, 10], F32)
        tmpw = const.tile([1, 40], F32)
        _emit_scalar_consts(
            nc, valsw, scal[0:1, 2:3], scal[0:1, 3:4], sw[0:1, 3:6], tmpw, d3, y3,
            eng=nc.gpsimd,
        )
        cbufw = const.tile([128, 10], F32)
        nc.gpsimd.partition_broadcast(cbufw[:], valsw[0:1, :])
        lhsT = [
            const.tile([128, C], F16, name=f"lhsT{k}", tag=f"lhsT{k}") for k in range(2)
        ]
        # m0 column halves of both k-chunks first: the first main-matmul
        # group reads only lhsT[k][:, 0:128]
        _emit_quant(
            nc, const, wtside[:, 0:128], cbufw, 128, 128,
            out=lhsT[0][:, 0:128], eng=nc.gpsimd, sfx="w0a", u_pre=uw[:, 0:128],
        )
        _emit_quant(
            nc, const, wtside[:, C : C + 128], cbufw, 128, 128,
            out=lhsT[1][:, 0:128], eng=nc.gpsimd, sfx="w1a", u_pre=uw[:, C : C + 128],
        )
        _emit_quant(
            nc, const, wtside[:, 128:256], cbufw, 128, 128,
            out=lhsT[0][:, 128:256], eng=nc.gpsimd, sfx="w0b", u_pre=uw[:, 128:256],
        )
        _emit_quant(
            nc, const, wtside[:, C + 128 : 2 * C], cbufw, 128, 128,
            out=lhsT[1][:, 128:256], eng=nc.gpsimd, sfx="w1b", u_pre=uw[:, C + 128 : 2 * C],
        )
        # correction weights (batch 0 only): quantize W[:, ch]^T directly.
        # fp16(quant) here equals the lhsT rows for those channels exactly.
        corrT = const.tile([NSEL, C], F16)
        _emit_quant(
            nc, const, wseltraw[:], cbufw[0:NSEL, :], NSEL, C,
            out=corrT[:], eng=nc.gpsimd, sfx="ws",
        )
        # K=64 zero-padded correction weights, batch-0 variants only:
        # corrT64[q] holds corrT rows at partition offset q*8. Full-height
        # tiles so base_partition is 0 (PE 64-row tiles need bases {0,64}).
        corrT64 = [
            const.tile([128, C], F16, name=f"corrT64_{q}", tag=f"corrT64_{q}")
            for q in range(4)
        ]
        for q in range(4):
            nc.gpsimd.memset(corrT64[q][0:64, :], 0.0)
            nc.scalar.dma_start(corrT64[q][q * 8 : q * 8 + NSEL, :], corrT[:])

    # ---- sel min/max: fused elementwise+reduce (TTR) on DVE ----
    # selp cols: 0 max(c0,c1), 1 max(c2,c3), 2 -min(c0,c1), 3 -min(c2,c3)
    selp = const.tile([128, 4], F32)
    for i in range(4):
        nc.vector.tensor_reduce(
            selp[:, i : i + 1], selredc[i][:], axis=AXIS.X, op=ALU.max
        )
    selpn = const.tile([128, 4], F32)
    for i in range(4):
        nc.vector.tensor_reduce(
            selpn[:, i : i + 1], selredc[i][:], axis=AXIS.X, op=ALU.min, negate=True
        )
    sc2 = const.tile([128, 2], F32)
    nc.vector.tensor_reduce(sc2[:, 0:1], selp[:, 0:4], axis=AXIS.X, op=ALU.max)
    nc.vector.tensor_reduce(sc2[:, 1:2], selpn[:, 0:4], axis=AXIS.X, op=ALU.max)
    gred = const.tile([128, 2], F32)
    nc.gpsimd.partition_all_reduce(
        gred[:, 0:2], sc2[:, 0:2], channels=128, reduce_op=bass_isa.ReduceOp.max
    )
    nc.vector.tensor_copy(scal[0:1, 0:1], gred[0:1, 0:1])
    nc.vector.tensor_scalar(scal[0:1, 1:2], gred[0:1, 1:2], -1.0, None, op0=ALU.mult)

    # ---- sel consts + activ (fp16) + batch-0 delta ----
    valss = const.tile([1, 10], F32)
    tmps = const.tile([1, 40], F32)
    _emit_scalar_consts(
        nc, valss, scal[0:1, 0:1], scal[0:1, 1:2], sw[0:1, 0:3], tmps, d3, y3
    )
    cbufs = const.tile([128, 10], F32)
    nc.gpsimd.partition_broadcast(cbufs[:], valss[0:1, :])
    activ = const.tile([128, 1024], F16)
    delta0 = const.tile([128, 1024], F16)
    nc.gpsimd.memset(delta0[32:64, :], 0.0)
    # column-split across DVE and gpsimd; batch-0 delta rows (0..31) come
    # from the same fp32 accumulator so the correction is fp16-consistent
    _emit_quant(
        nc, const, selloc[:, 0:512], cbufs, 128, 512,
        out=activ[:, 0:512], eng=nc.vector, sfx="sa",
        delta_out=delta0[:, 0:512], delta_src=selloc[:, 0:512], delta_rows=32,
    )
    _emit_quant(
        nc, const, selloc[:, 512:1024], cbufs, 128, 512,
        out=activ[:, 512:1024], eng=nc.gpsimd, sfx="sb",
        delta_out=delta0[:, 512:1024], delta_src=selloc[:, 512:1024], delta_rows=32,
    )

    # ---- main loop ----
    for b in range(BPC):
        rhs0 = rhs_pool.tile([128, HW], F16, tag="rhs0")
        nc.sync.dma_start(rhs0[:], x_ap[b, 0:128, :])
        rhs1 = rhs_pool.tile([128, HW], F16, tag="rhs1")
        nc.sync.dma_start(rhs1[:], x_ap[b, 128:256, :])
        if b > 0:
            # patch the 8 sel rows with the quantized activations
            for q in range(4):
                p0 = b * 32 + q * 8
                nc.scalar.dma_start(
                    rhs1[SELP0 : SELP0 + NSEL, q * 1024 : (q + 1) * 1024],
                    activ[p0 : p0 + NSEL, :],
                )
        for m in range(2):
            outsb = out_pool.tile([128, HW], F16, name="outsb", tag="outsb")
            for g in range(4):  # two [128,512] PSUM tiles (1 bank each) per group
                gi = ((b * 2 + m) * 4 + g)
                pts = []
                for h in range(2):
                    n = g * 2 + h
                    pt = psB.tile([128, 512], F32, name="ptile", tag="ptile")
                    pts.append(pt)
                    nc.tensor.matmul(
                        pt[:],
                        lhsT[0][:, m * 128 : (m + 1) * 128],
                        rhs0[:, n * 512 : (n + 1) * 512],
                        start=True,
                        stop=False,
                    )
                    nc.tensor.matmul(
                        pt[:],
                        lhsT[1][:, m * 128 : (m + 1) * 128],
                        rhs1[:, n * 512 : (n + 1) * 512],
                        start=False,
                        stop=(b > 0),
                    )
                    if b == 0:
                        q, r = divmod(n, 2)
                        nc.tensor.matmul(
                            pt[:],
                            corrT64[q][0:64, m * 128 : (m + 1) * 128],
                            delta0[0:64, r * 512 : (r + 1) * 512],
                            start=False,
                            stop=True,
                        )
                # eviction with fp32 -> fp16 cast: alternate ACT/DVE
                for h in range(2):
                    n = g * 2 + h
                    if (gi + h) % 2 == 0:
                        nc.scalar.copy(outsb[:, n * 512 : (n + 1) * 512], pts[h][:])
                    else:
                        nc.vector.tensor_copy(
                            outsb[:, n * 512 : (n + 1) * 512], pts[h][:]
                        )
            is_last = b == BPC - 1 and m == 1
            if is_last:
                # final drain per eviction so the tail stays short
                for g in range(4):
                    eng = (nc.scalar, nc.gpsimd, nc.sync, nc.gpsimd)[g % 4]
                    eng.dma_start(
                        out_ap[b, m * 128 : (m + 1) * 128, g * 1024 : (g + 1) * 1024],
                        outsb[:, g * 1024 : (g + 1) * 1024],
                    )
            else:
                # one big async out-DMA per (b, m) on SWDGE
                nc.gpsimd.dma_start(out_ap[b, m * 128 : (m + 1) * 128, :], outsb[:])


def build_program(ch):
    nc = bacc.Bacc(
        "TRN2", target_bir_lowering=False, debug=False, num_devices=NCORES
    )
    x_t = nc.dram_tensor("x", [BPC, C, HW], F16, kind="ExternalInput").ap()
    selred_t = nc.dram_tensor("selred", [128, 8192], F32, kind="ExternalInput").ap()
    selloc_t = nc.dram_tensor("selloc", [128, 1024], F32, kind="ExternalInput").ap()
    w_t = nc.dram_tensor("wt", [C, C], F32, kind="ExternalInput").ap()
    ws_t = nc.dram_tensor("wselt", [NSEL, C], F32, kind="ExternalInput").ap()
    al_t = nc.dram_tensor("alphas", [1, 6], F32, kind="ExternalInput").ap()
    out_t = nc.dram_tensor("out", [BPC, C, HW], F16, kind="ExternalOutput").ap()
    with tile.TileContext(nc) as tc:
        with ExitStack() as ctx:
            _kernel_body(
                ctx, tc, ch, x_t, selred_t, selloc_t, w_t, ws_t, al_t, out_t
            )
    nc.compile()
    return nc


def make_in_maps(x, alpha_activ, alpha_weight, conv_weight, selected_channels):
    x = np.ascontiguousarray(np.asarray(x, dtype=np.float32).reshape(B, C, HW))
    ch = [int(v) for v in np.asarray(selected_channels).ravel()]
    sel = np.ascontiguousarray(x[:, ch, :])  # [32, 8, 4096]
    selred = sel.reshape(128, 8192)
    alphas = np.concatenate(
        [np.asarray(alpha_activ).ravel(), np.asarray(alpha_weight).ravel()]
    ).astype(np.float32).reshape(1, 6)
    wmat = np.asarray(conv_weight, dtype=np.float32).reshape(C, C)
    # channel permutation: all sel channels at positions 248..255 (k-chunk-1
    # partitions 120..127); wt rows permuted to match (the contraction is
    # permutation-invariant)
    nonsel = [c for c in range(C) if c not in set(ch)]
    perm = nonsel + ch  # position i holds original channel perm[i]
    xp = x[:, perm, :]
    wt = np.ascontiguousarray(wmat.T[perm, :])
    wselt = np.ascontiguousarray(wmat[:, ch].T)  # [8, 256]
    in_maps = []
    for c in range(NCORES):
        xs = np.ascontiguousarray(xp[c * BPC : (c + 1) * BPC].astype(np.float16))
        # selloc layout: partition p = b*32 + q*8 + j holds
        # sel[core*4+b, j, q*1024 : (q+1)*1024]
        sl = sel[c * BPC : (c + 1) * BPC].reshape(BPC, NSEL, 4, 1024)
        selloc = np.ascontiguousarray(sl.transpose(0, 2, 1, 3).reshape(128, 1024))
        in_maps.append(
            {
                "x": xs,
                "selred": selred,
                "selloc": selloc,
                "wt": wt,
                "wselt": wselt,
                "alphas": alphas,
            }
        )
    return ch, in_maps


def kernel(x, alpha_activ, alpha_weight, conv_weight, selected_channels):
    from concourse.bass_utils import run_bass_kernel_spmd

    ch, in_maps = make_in_maps(
        x, alpha_activ, alpha_weight, conv_weight, selected_channels
    )
    nc = build_program(ch)
    res = run_bass_kernel_spmd(nc, in_maps, core_ids=list(range(NCORES)))
    outs = [
        res.results[c]["out"].astype(np.float32).reshape(BPC, C, H, W)
        for c in range(NCORES)
    ]
    return np.concatenate(outs, axis=0)


# revision 39
# speedup vs baseline: 1.0453x; 1.0453x over previous
"""Trainium2 Bass kernel for MixActivConv2d (mixed-precision fake-quant + 1x1 conv).

Reference computation:
  sel = x[:, ch]                                   # gather 8 channels
  activ = sum_i softmax(aa)[i] * uq(sel, bit_i)    # global-minmax fake quant
  x_q = x with sel channels replaced by activ
  w_q = sum_i softmax(aw)[i] * uq(w, bit_i)
  out = conv1x1(x_q, w_q)  ==  w_q[256,256] @ x_q[b, 256, 4096]

Strategy (8 cores, data-parallel over batch, 4 batches/core):
  - fp16 matmul datapath: x streams in fp16 (half the HBM read), the
    exactly-quantized weights/activations are written as fp16, PSUM (fp32)
    evicts with an fp16 cast (half the HBM write), host upcasts.
    Quantization bucket decisions stay bit-exact fp32 (a flipped 2-bit
    bucket near a tie would cost ~0.05 rel err); the fp16 rounding after
    quantization only adds ~1e-3 smooth error vs the 2e-2 gate.
  - host permutes the channel axis so the 8 selected channels sit at
    partitions 120..127 of the k-chunk-1 rhs tile (wt rows permuted to
    match; conv is permutation-invariant in the contraction index).
  - batches 1..3: the quantized activations are DMA-scattered over the
    raw rows of the loaded rhs tile (4 rectangular SBUF->SBUF DMAs per
    batch), so their matmuls directly produce the final output.
  - batch 0 instead keeps a rank-8 PSUM correction (main matmuls on raw
    x + K=64 corr matmuls with activ-x), hiding the global min/max
    reduction latency behind batch 0's main matmuls.
  - global sel min/max from a replicated copy of the gathered channels
    (4 MB), reduced on-device on every core (no collectives); fused
    tensor_tensor_reduce halves the reduction passes.
  - rounding via the fp32 magic-number trick (round-to-nearest-even,
    matching jnp.round)
"""

import sys
from contextlib import ExitStack

import numpy as np

sys.path.insert(0, "/opt/trn_rl_repo")

import concourse.bass as bass  # noqa: E402
import concourse.mybir as mybir  # noqa: E402
import concourse.tile as tile  # noqa: E402
from concourse import bacc  # noqa: E402

NCORES = 8
B, C, H, W = 32, 256, 64, 64
HW = H * W  # 4096
BPC = B // NCORES  # batches per core = 4
NSEL = 8
QMAX = (3.0, 15.0, 255.0)  # 2^bit - 1 for bits (2, 4, 8)
MAGIC = 12582912.0  # 1.5 * 2**23: x + MAGIC - MAGIC == rne-round(x) for |x| < 2^22
F32 = mybir.dt.float32
F16 = mybir.dt.float16
ALU = mybir.AluOpType
AXIS = mybir.AxisListType
ACTF = mybir.ActivationFunctionType
SELP0 = 120  # sel channels live at partitions 120..127 of k-chunk 1


def _emit_scalar_consts(nc, vals, scal_mx, scal_mn, sw, tmp, d3, y3, eng=None):
    """Scalar chain on partition 0. Writes vals [1,10]:
    cols 0..2 inv_i (=1/scale_i), 3..5 k_i (=sw_i*scale_i), 6 mn, 7 MAGIC.

    scale_i = fp32-exact (mx-mn)/qmax_i via one Newton step with an exact
    (Dekker) residual: the divisors fit in 12 bits so their Veltkamp low
    split is zero and every product in the error term is exact. Verified
    bit-identical to IEEE fp32 division over millions of samples.
    d3/y3: [1,3] const tiles holding qmax_i and fl(1/qmax_i).
    tmp is a [1, 40] scratch tile.
    """

    eng = eng if eng is not None else nc.vector

    def col3(j):
        return tmp[0:1, j : j + 3]

    rng = tmp[0:1, 36:37]
    eng.tensor_sub(rng, scal_mx, scal_mn)
    n_b = rng.to_broadcast((1, 3))
    q0, p, ca, t1, ah, al, t2, t3, t4, e, t5, r = (col3(3 * j) for j in range(12))
    eng.tensor_mul(q0, n_b, y3)
    eng.tensor_mul(p, q0, d3)
    eng.tensor_scalar(ca, q0, 4097.0, None, op0=ALU.mult)
    eng.tensor_sub(t1, ca, q0)
    eng.tensor_sub(ah, ca, t1)
    eng.tensor_sub(al, q0, ah)
    eng.tensor_mul(t2, ah, d3)
    eng.tensor_sub(t3, t2, p)
    eng.tensor_mul(t4, al, d3)
    eng.tensor_add(e, t3, t4)
    eng.tensor_sub(t5, n_b, p)
    eng.tensor_sub(r, t5, e)
    scale3 = col3(0)  # reuse q0's slot via separate name for clarity
    eng.tensor_mul(t2, r, y3)  # t2 = r*y
    eng.tensor_add(scale3, q0, t2)  # scale3 overwrites q0 in place
    # inv_i = 1/scale_i (bit-exact reciprocal); k_i = sw_i * scale_i
    recip_inst = nc.vector.reciprocal(vals[0:1, 0:3], scale3)
    eng.tensor_mul(vals[0:1, 3:6], scale3, sw)
    eng.tensor_copy(vals[0:1, 6:7], scal_mn)
    eng.memset(vals[0:1, 7:8], MAGIC)
    return recip_inst


def _emit_quant(nc, pool, src, cbuf, nparts, nfree, out=None, eng=None, sfx="",
                u_pre=None, delta_out=None, delta_src=None, delta_rows=0):
    """Emit the 3-bit blended fake-quant of src [nparts, nfree].

    u = src - mn
    r_i = u*inv_i + MAGIC          (the fp32 add rounds to integer, RNE)
    p_i = (r_i - MAGIC) * k_i      (subtract is exact, result = round(u/scale)*k)
    out = p0 + p1 + p2 + mn        (written to `out`, any dtype)
    If delta_out given: also delta_out[0:delta_rows] =
        (acc + mn) - delta_src   on the first delta_rows partitions.
    Returns the output tile ([nparts, nfree]).
    """
    eng = eng if eng is not None else nc.vector
    if u_pre is not None:
        u = u_pre
    else:
        u = pool.tile([nparts, nfree], F32, tag=f"qu_{nparts}_{nfree}{sfx}", name="qu")
        eng.tensor_scalar(u, src, cbuf[:, 6:7], None, op0=ALU.subtract)
    p = []
    for i in range(3):
        # all on DVE/Pool, in place: per-op IEEE fp32 rounding must match the
        # reference's separate mul/add ops (ACT's fused internal arithmetic
        # flips near-tie elements into the next quant bucket on HW)
        pi = pool.tile(
            [nparts, nfree], F32, tag=f"ptmp{i}_{nparts}_{nfree}{sfx}", name=f"ptmp{i}"
        )
        eng.tensor_scalar(pi, u, cbuf[:, i : i + 1], None, op0=ALU.mult)
        eng.tensor_scalar(pi, pi, MAGIC, None, op0=ALU.add)
        eng.tensor_scalar(
            pi, pi, MAGIC, cbuf[:, 3 + i : 4 + i], op0=ALU.subtract, op1=ALU.mult
        )
        p.append(pi)
    eng.tensor_add(p[0], p[0], p[1])
    eng.tensor_add(p[0], p[0], p[2])
    outt = out if out is not None else pool.tile(
        [nparts, nfree], F32, tag=f"qout_{nparts}_{nfree}{sfx}", name="qout"
    )
    eng.tensor_scalar(outt, p[0], cbuf[:, 6:7], None, op0=ALU.add)
    if delta_out is not None:
        nc.vector.scalar_tensor_tensor(
            delta_out[0:delta_rows, :],
            p[0][0:delta_rows, :],
            cbuf[0:delta_rows, 6:7],
            delta_src[0:delta_rows, :],
            op0=ALU.add,
            op1=ALU.subtract,
        )
    return outt


def _kernel_body(ctx, tc, ch, x_ap, selred_ap, selloc_ap, w_ap, ws_ap, al_ap, out_ap):
    nc = tc.nc
    import concourse.bass_isa as bass_isa

    const = ctx.enter_context(tc.tile_pool(name="const", bufs=1))
    rhs_pool = ctx.enter_context(tc.tile_pool(name="rhs", bufs=2))
    out_pool = ctx.enter_context(tc.tile_pool(name="outsb", bufs=2))
    psB = ctx.enter_context(tc.tile_pool(name="psB", bufs=8, space="PSUM"))

    # ---- inputs ----
    alphas = const.tile([1, 6], F32)
    nc.gpsimd.dma_start(alphas[:], al_ap)
    # W arrives pre-transposed and channel-permuted from the host
    wtside = const.tile([128, 2 * C], F32)  # W^T chunks side by side
    nc.sync.dma_start(wtside[:, 0:C], w_ap[0:128, :])
    nc.sync.dma_start(wtside[:, C : 2 * C], w_ap[128:256, :])
    wseltraw = const.tile([NSEL, C], F32)
    nc.sync.dma_start(wseltraw[:], ws_ap)
    # replicated gathered-channel copy for the global min/max.  A DMA
    # occupies its issuing queue for the whole transfer, so spread: c0/c1
    # on ACT (idle until evictions), c2 on SP before the batch-0 rhs
    # stream, c3 on SP right after it.  Pool stays clear for the W path
    # (which gates the first matmul).
    selredc = [
        const.tile([128, 2048], F32, name=f"selredc{i}", tag=f"selredc{i}")
        for i in range(4)
    ]
    nc.sync.dma_start(selredc[2][:], selred_ap[:, 2 * 2048 : 3 * 2048])
    nc.scalar.dma_start(selredc[0][:], selred_ap[:, 0:2048])
    nc.scalar.dma_start(selredc[1][:], selred_ap[:, 2048 : 2 * 2048])
    selloc = const.tile([128, 1024], F32)
    nc.scalar.dma_start(selloc[:], selloc_ap)
    # batch-0 rhs prefetch, then the last min/max chunk rides SP behind it
    rhs_b0 = []
    for k in range(2):
        t = rhs_pool.tile([128, HW], F16, tag=f"rhs{k}")
        nc.sync.dma_start(t[:], x_ap[0, k * 128 : (k + 1) * 128, :])
        rhs_b0.append(t)
    nc.sync.dma_start(selredc[3][:], selred_ap[:, 3 * 2048 : 4 * 2048])

    with tc.high_priority():
        # ---- softmax of both alpha vectors (on partition 0) ----
        ex = const.tile([1, 6], F32)
        nc.scalar.activation(ex[:], alphas[:], ACTF.Exp)
        sums = const.tile([1, 8], F32)
        nc.vector.tensor_reduce(sums[0:1, 0:1], ex[0:1, 0:3], axis=AXIS.X, op=ALU.add)
        nc.vector.tensor_reduce(sums[0:1, 1:2], ex[0:1, 3:6], axis=AXIS.X, op=ALU.add)
        nc.vector.reciprocal(sums[0:1, 2:3], sums[0:1, 0:1])
        nc.vector.reciprocal(sums[0:1, 3:4], sums[0:1, 1:2])
        sw = const.tile([1, 6], F32)  # cols 0..2 = sw_activ, 3..5 = sw_weight
        nc.vector.tensor_scalar(sw[0:1, 0:3], ex[0:1, 0:3], sums[0:1, 2:3], None, op0=ALU.mult)
        sw_last = nc.vector.tensor_scalar(
            sw[0:1, 3:6], ex[0:1, 3:6], sums[0:1, 3:4], None, op0=ALU.mult
        )

        # qmax and fl(1/qmax) constant vectors for the exact-division sequence
        d3 = const.tile([1, 3], F32)
        y3 = const.tile([1, 3], F32)
        for i, qm in enumerate(QMAX):
            nc.gpsimd.memset(d3[0:1, i : i + 1], float(qm))
            nc.gpsimd.memset(y3[0:1, i : i + 1], float(np.float32(1.0) / np.float32(qm)))

        # ---- W min/max: DVE free-axis partials + gpsimd cross-partition ----
        scal = const.tile([1, 8], F32)  # 0 smx, 1 smn, 2 wmx, 3 wmn
        wp = const.tile([128, 2], F32)
        wg = const.tile([128, 2], F32)
        nc.vector.tensor_reduce(wp[:, 0:1], wtside[:], axis=AXIS.X, op=ALU.max)
        nc.vector.tensor_reduce(
            wp[:, 1:2], wtside[:], axis=AXIS.X, op=ALU.min, negate=True
        )
        nc.gpsimd.partition_all_reduce(
            wg[:, 0:2], wp[:, 0:2], channels=128, reduce_op=bass_isa.ReduceOp.max
        )
        nc.gpsimd.tensor_copy(scal[0:1, 2:3], wg[0:1, 0:1])
        nc.gpsimd.tensor_scalar(scal[0:1, 3:4], wg[0:1, 1:2], -1.0, None, op0=ALU.mult)
        mnbw = const.tile([128, 1], F32)
        nc.gpsimd.partition_broadcast(mnbw[:], scal[0:1, 3:4])
        uw = const.tile([128, 2 * C], F32)
        nc.gpsimd.tensor_scalar(uw[:], wtside[:], mnbw[:, 0:1], None, op0=ALU.subtract)

        # ---- W consts + quantized weights (fp16 lhsT) ----
        valsw = const.tile([1, 10], F32)
        tmpw = const.tile([1, 40], F32)
        recip_w = _emit_scalar_consts(
            nc, valsw, scal[0:1, 2:3], scal[0:1, 3:4], sw[0:1, 3:6], tmpw, d3, y3,
            eng=nc.gpsimd,
        )
        cbufw = const.tile([128, 10], F32)
        nc.gpsimd.partition_broadcast(cbufw[:], valsw[0:1, :])
        lhsT = [
            const.tile([128, C], F16, name=f"lhsT{k}", tag=f"lhsT{k}") for k in range(2)
        ]
        # m0 column halves of both k-chunks first: the first main-matmul
        # group reads only lhsT[k][:, 0:128]
        _emit_quant(
            nc, const, wtside[:, 0:128], cbufw, 128, 128,
            out=lhsT[0][:, 0:128], eng=nc.gpsimd, sfx="w0a", u_pre=uw[:, 0:128],
        )
        _emit_quant(
            nc, const, wtside[:, C : C + 128], cbufw, 128, 128,
            out=lhsT[1][:, 0:128], eng=nc.gpsimd, sfx="w1a", u_pre=uw[:, C : C + 128],
        )
        _emit_quant(
            nc, const, wtside[:, 128:256], cbufw, 128, 128,
            out=lhsT[0][:, 128:256], eng=nc.gpsimd, sfx="w0b", u_pre=uw[:, 128:256],
        )
        _emit_quant(
            nc, const, wtside[:, C + 128 : 2 * C], cbufw, 128, 128,
            out=lhsT[1][:, 128:256], eng=nc.gpsimd, sfx="w1b", u_pre=uw[:, C + 128 : 2 * C],
        )
        # correction weights (batch 0 only): quantize W[:, ch]^T directly.
        # fp16(quant) here equals the lhsT rows for those channels exactly.
        corrT = const.tile([NSEL, C], F16)
        _emit_quant(
            nc, const, wseltraw[:], cbufw[0:NSEL, :], NSEL, C,
            out=corrT[:], eng=nc.gpsimd, sfx="ws",
        )
        # K=64 zero-padded correction weights, batch-0 variants only:
        # corrT64[q] holds corrT rows at partition offset q*8. Full-height
        # tiles so base_partition is 0 (PE 64-row tiles need bases {0,64}).
        corrT64 = [
            const.tile([128, C], F16, name=f"corrT64_{q}", tag=f"corrT64_{q}")
            for q in range(4)
        ]
        for q in range(4):
            nc.gpsimd.memset(corrT64[q][0:64, :], 0.0)
            nc.scalar.dma_start(corrT64[q][q * 8 : q * 8 + NSEL, :], corrT[:])

    # ---- sel min/max: fused elementwise+reduce (TTR) on DVE ----
    # selp cols: 0 max(c0,c1), 1 max(c2,c3), 2 -min(c0,c1), 3 -min(c2,c3)
    # plain per-chunk partials on DVE (TTR fused reduce crashes TRN2 HW
    # here), in chunk-arrival order. The explicit deps force the tiny DVE
    # ops of the weights path (softmax tail, reciprocal) ahead of these
    # long reductions in the static DVE order — otherwise the scheduler's
    # criticality heuristic starves the W pipeline for ~14us.
    from concourse.tile import add_dep_helper

    selp = const.tile([128, 8], F32)
    for i in (2, 0, 1, 3):
        rmax = nc.vector.tensor_reduce(
            selp[:, i : i + 1], selredc[i][:], axis=AXIS.X, op=ALU.max
        )
        rmin = nc.vector.tensor_reduce(
            selp[:, 4 + i : 5 + i], selredc[i][:], axis=AXIS.X, op=ALU.min,
            negate=True,
        )
        for r in (rmax, rmin):
            add_dep_helper(r.ins, recip_w.ins, reason="W consts before sel reductions")
            add_dep_helper(r.ins, sw_last.ins, reason="softmax tail before sel reductions")
    sc2 = const.tile([128, 2], F32)
    nc.vector.tensor_reduce(sc2[:, 0:1], selp[:, 0:4], axis=AXIS.X, op=ALU.max)
    nc.vector.tensor_reduce(sc2[:, 1:2], selp[:, 4:8], axis=AXIS.X, op=ALU.max)
    gred = const.tile([128, 2], F32)
    nc.gpsimd.partition_all_reduce(
        gred[:, 0:2], sc2[:, 0:2], channels=128, reduce_op=bass_isa.ReduceOp.max
    )
    nc.vector.tensor_copy(scal[0:1, 0:1], gred[0:1, 0:1])
    nc.vector.tensor_scalar(scal[0:1, 1:2], gred[0:1, 1:2], -1.0, None, op0=ALU.mult)

    # ---- sel consts + activ (fp16) + batch-0 delta ----
    valss = const.tile([1, 10], F32)
    tmps = const.tile([1, 40], F32)
    _emit_scalar_consts(
        nc, valss, scal[0:1, 0:1], scal[0:1, 1:2], sw[0:1, 0:3], tmps, d3, y3
    )
    cbufs = const.tile([128, 10], F32)
    nc.gpsimd.partition_broadcast(cbufs[:], valss[0:1, :])
    activ = const.tile([128, 1024], F16)
    delta0 = const.tile([128, 1024], F16)
    nc.gpsimd.memset(delta0[32:64, :], 0.0)
    # column-split across DVE and gpsimd; batch-0 delta rows (0..31) come
    # from the same fp32 accumulator so the correction is fp16-consistent
    _emit_quant(
        nc, const, selloc[:, 0:512], cbufs, 128, 512,
        out=activ[:, 0:512], eng=nc.vector, sfx="sa",
        delta_out=delta0[:, 0:512], delta_src=selloc[:, 0:512], delta_rows=32,
    )
    _emit_quant(
        nc, const, selloc[:, 512:1024], cbufs, 128, 512,
        out=activ[:, 512:1024], eng=nc.gpsimd, sfx="sb",
        delta_out=delta0[:, 512:1024], delta_src=selloc[:, 512:1024], delta_rows=32,
    )

    # ---- main loop ----
    for b in range(BPC):
        if b == 0:
            rhs0, rhs1 = rhs_b0
        else:
            rhs0 = rhs_pool.tile([128, HW], F16, tag="rhs0")
            nc.sync.dma_start(rhs0[:], x_ap[b, 0:128, :])
            rhs1 = rhs_pool.tile([128, HW], F16, tag="rhs1")
            nc.sync.dma_start(rhs1[:], x_ap[b, 128:256, :])
        if b > 0:
            # patch the 8 sel rows with the quantized activations
            for q in range(4):
                p0 = b * 32 + q * 8
                nc.scalar.dma_start(
                    rhs1[SELP0 : SELP0 + NSEL, q * 1024 : (q + 1) * 1024],
                    activ[p0 : p0 + NSEL, :],
                )
        for m in range(2):
            outsb = out_pool.tile([128, HW], F16, name="outsb", tag="outsb")
            for g in range(4):  # two [128,512] PSUM tiles (1 bank each) per group
                gi = ((b * 2 + m) * 4 + g)
                pts = []
                for h in range(2):
                    n = g * 2 + h
                    pt = psB.tile([128, 512], F32, name="ptile", tag="ptile")
                    pts.append(pt)
                    nc.tensor.matmul(
                        pt[:],
                        lhsT[0][:, m * 128 : (m + 1) * 128],
                        rhs0[:, n * 512 : (n + 1) * 512],
                        start=True,
                        stop=False,
                    )
                    nc.tensor.matmul(
                        pt[:],
                        lhsT[1][:, m * 128 : (m + 1) * 128],
                        rhs1[:, n * 512 : (n + 1) * 512],
                        start=False,
                        stop=(b > 0),
                    )
                    if b == 0:
                        q, r = divmod(n, 2)
                        nc.tensor.matmul(
                            pt[:],
                            corrT64[q][0:64, m * 128 : (m + 1) * 128],
                            delta0[0:64, r * 512 : (r + 1) * 512],
                            start=False,
                            stop=True,
                        )
                # eviction with fp32 -> fp16 cast: alternate ACT/DVE
                for h in range(2):
                    n = g * 2 + h
                    if (gi + h) % 2 == 0:
                        nc.scalar.copy(outsb[:, n * 512 : (n + 1) * 512], pts[h][:])
                    else:
                        nc.vector.tensor_copy(
                            outsb[:, n * 512 : (n + 1) * 512], pts[h][:]
                        )
            is_last = b == BPC - 1 and m == 1
            if is_last:
                # final drain per eviction so the tail stays short
                for g in range(4):
                    eng = (nc.scalar, nc.gpsimd, nc.sync, nc.gpsimd)[g % 4]
                    eng.dma_start(
                        out_ap[b, m * 128 : (m + 1) * 128, g * 1024 : (g + 1) * 1024],
                        outsb[:, g * 1024 : (g + 1) * 1024],
                    )
            else:
                # one big out-DMA per (b, m), rotated across Pool/ACT/SP
                eng = (nc.gpsimd, nc.scalar, nc.gpsimd, nc.sync)[(b * 2 + m) % 4]
                eng.dma_start(out_ap[b, m * 128 : (m + 1) * 128, :], outsb[:])


def build_program(ch):
    nc = bacc.Bacc(
        "TRN2", target_bir_lowering=False, debug=False, num_devices=NCORES
    )
    x_t = nc.dram_tensor("x", [BPC, C, HW], F16, kind="ExternalInput").ap()
    selred_t = nc.dram_tensor("selred", [128, 8192], F32, kind="ExternalInput").ap()
    selloc_t = nc.dram_tensor("selloc", [128, 1024], F32, kind="ExternalInput").ap()
    w_t = nc.dram_tensor("wt", [C, C], F32, kind="ExternalInput").ap()
    ws_t = nc.dram_tensor("wselt", [NSEL, C], F32, kind="ExternalInput").ap()
    al_t = nc.dram_tensor("alphas", [1, 6], F32, kind="ExternalInput").ap()
    out_t = nc.dram_tensor("out", [BPC, C, HW], F16, kind="ExternalOutput").ap()
    with tile.TileContext(nc) as tc:
        with ExitStack() as ctx:
            _kernel_body(
                ctx, tc, ch, x_t, selred_t, selloc_t, w_t, ws_t, al_t, out_t
            )
    nc.compile()
    return nc


def make_in_maps(x, alpha_activ, alpha_weight, conv_weight, selected_channels):
    x = np.ascontiguousarray(np.asarray(x, dtype=np.float32).reshape(B, C, HW))
    ch = [int(v) for v in np.asarray(selected_channels).ravel()]
    sel = np.ascontiguousarray(x[:, ch, :])  # [32, 8, 4096]
    selred = sel.reshape(128, 8192)
    alphas = np.concatenate(
        [np.asarray(alpha_activ).ravel(), np.asarray(alpha_weight).ravel()]
    ).astype(np.float32).reshape(1, 6)
    wmat = np.asarray(conv_weight, dtype=np.float32).reshape(C, C)
    # channel permutation: all sel channels at positions 248..255 (k-chunk-1
    # partitions 120..127); wt rows permuted to match (the contraction is
    # permutation-invariant)
    nonsel = [c for c in range(C) if c not in set(ch)]
    perm = nonsel + ch  # position i holds original channel perm[i]
    xp = x[:, perm, :]
    wt = np.ascontiguousarray(wmat.T[perm, :])
    wselt = np.ascontiguousarray(wmat[:, ch].T)  # [8, 256]
    in_maps = []
    for c in range(NCORES):
        xs = np.ascontiguousarray(xp[c * BPC : (c + 1) * BPC].astype(np.float16))
        # selloc layout: partition p = b*32 + q*8 + j holds
        # sel[core*4+b, j, q*1024 : (q+1)*1024]
        sl = sel[c * BPC : (c + 1) * BPC].reshape(BPC, NSEL, 4, 1024)
        selloc = np.ascontiguousarray(sl.transpose(0, 2, 1, 3).reshape(128, 1024))
        in_maps.append(
            {
                "x": xs,
                "selred": selred,
                "selloc": selloc,
                "wt": wt,
                "wselt": wselt,
                "alphas": alphas,
            }
        )
    return ch, in_maps


def kernel(x, alpha_activ, alpha_weight, conv_weight, selected_channels):
    from concourse.bass_utils import run_bass_kernel_spmd

    ch, in_maps = make_in_maps(
        x, alpha_activ, alpha_weight, conv_weight, selected_channels
    )
    nc = build_program(ch)
    res = run_bass_kernel_spmd(nc, in_maps, core_ids=list(range(NCORES)))
    outs = [
        res.results[c]["out"].astype(np.float32).reshape(BPC, C, H, W)
        for c in range(NCORES)
    ]
    return np.concatenate(outs, axis=0)


# revision 50
# speedup vs baseline: 1.2425x; 1.1886x over previous
"""Trainium2 Bass kernel for MixActivConv2d (mixed-precision fake-quant + 1x1 conv).

Reference computation:
  sel = x[:, ch]                                   # gather 8 channels
  activ = sum_i softmax(aa)[i] * uq(sel, bit_i)    # global-minmax fake quant
  x_q = x with sel channels replaced by activ
  w_q = sum_i softmax(aw)[i] * uq(w, bit_i)
  out = conv1x1(x_q, w_q)  ==  w_q[256,256] @ x_q[b, 256, 4096]

Strategy (8 cores, data-parallel over batch, 4 batches/core):
  - fp16 matmul datapath: x streams in fp16 (half the HBM read), the
    exactly-quantized weights/activations are written as fp16, PSUM (fp32)
    evicts with an fp16 cast (half the HBM write), host upcasts.
    Quantization bucket decisions stay bit-exact fp32 (a flipped 2-bit
    bucket near a tie would cost ~0.05 rel err); the fp16 rounding after
    quantization only adds ~1e-3 smooth error vs the 2e-2 gate.
  - host permutes the channel axis so the 8 selected channels sit at
    partitions 120..127 of the k-chunk-1 rhs tile (wt rows permuted to
    match; conv is permutation-invariant in the contraction index).
  - batches 1..3: the quantized activations are DMA-scattered over the
    raw rows of the loaded rhs tile (4 rectangular SBUF->SBUF DMAs per
    batch), so their matmuls directly produce the final output.
  - batch 0 instead keeps a rank-8 PSUM correction (main matmuls on raw
    x + K=64 corr matmuls with activ-x), hiding the global min/max
    reduction latency behind batch 0's main matmuls.
  - global sel min/max from a replicated copy of the gathered channels
    (4 MB), reduced on-device on every core (no collectives); fused
    tensor_tensor_reduce halves the reduction passes.
  - rounding via the fp32 magic-number trick (round-to-nearest-even,
    matching jnp.round)
"""

import sys
from contextlib import ExitStack

import numpy as np

sys.path.insert(0, "/opt/trn_rl_repo")

import concourse.bass as bass  # noqa: E402
import concourse.mybir as mybir  # noqa: E402
import concourse.tile as tile  # noqa: E402
from concourse import bacc  # noqa: E402

NCORES = 8
B, C, H, W = 32, 256, 64, 64
HW = H * W  # 4096
BPC = B // NCORES  # batches per core = 4
NSEL = 8
QMAX = (3.0, 15.0, 255.0)  # 2^bit - 1 for bits (2, 4, 8)
MAGIC = 12582912.0  # 1.5 * 2**23: x + MAGIC - MAGIC == rne-round(x) for |x| < 2^22
F32 = mybir.dt.float32
F16 = mybir.dt.float16
ALU = mybir.AluOpType
AXIS = mybir.AxisListType
ACTF = mybir.ActivationFunctionType
SELP0 = 120  # sel channels live at partitions 120..127 of k-chunk 1


def _emit_scalar_consts(nc, vals, scal_mx, scal_mn, sw, tmp, d3, y3, eng=None):
    """Scalar chain on partition 0. Writes vals [1,10]:
    cols 0..2 inv_i (=1/scale_i), 3..5 k_i (=sw_i*scale_i), 6 mn, 7 MAGIC.

    scale_i = fp32-exact (mx-mn)/qmax_i via one Newton step with an exact
    (Dekker) residual: the divisors fit in 12 bits so their Veltkamp low
    split is zero and every product in the error term is exact. Verified
    bit-identical to IEEE fp32 division over millions of samples.
    d3/y3: [1,3] const tiles holding qmax_i and fl(1/qmax_i).
    tmp is a [1, 40] scratch tile.
    """

    eng = eng if eng is not None else nc.vector

    def col3(j):
        return tmp[0:1, j : j + 3]

    rng = tmp[0:1, 36:37]
    eng.tensor_sub(rng, scal_mx, scal_mn)
    n_b = rng.to_broadcast((1, 3))
    q0, p, ca, t1, ah, al, t2, t3, t4, e, t5, r = (col3(3 * j) for j in range(12))
    eng.tensor_mul(q0, n_b, y3)
    eng.tensor_mul(p, q0, d3)
    eng.tensor_scalar(ca, q0, 4097.0, None, op0=ALU.mult)
    eng.tensor_sub(t1, ca, q0)
    eng.tensor_sub(ah, ca, t1)
    eng.tensor_sub(al, q0, ah)
    eng.tensor_mul(t2, ah, d3)
    eng.tensor_sub(t3, t2, p)
    eng.tensor_mul(t4, al, d3)
    eng.tensor_add(e, t3, t4)
    eng.tensor_sub(t5, n_b, p)
    eng.tensor_sub(r, t5, e)
    scale3 = col3(0)  # reuse q0's slot via separate name for clarity
    eng.tensor_mul(t2, r, y3)  # t2 = r*y
    eng.tensor_add(scale3, q0, t2)  # scale3 overwrites q0 in place
    # inv_i = 1/scale_i (bit-exact reciprocal); k_i = sw_i * scale_i
    recip_inst = nc.vector.reciprocal(vals[0:1, 0:3], scale3)
    eng.tensor_mul(vals[0:1, 3:6], scale3, sw)
    eng.tensor_copy(vals[0:1, 6:7], scal_mn)
    eng.memset(vals[0:1, 7:8], MAGIC)
    return recip_inst


def _emit_quant(nc, pool, src, cbuf, nparts, nfree, out=None, eng=None, sfx="",
                u_pre=None, delta_out=None, delta_src=None, delta_rows=0):
    """Emit the 3-bit blended fake-quant of src [nparts, nfree].

    u = src - mn
    r_i = u*inv_i + MAGIC          (the fp32 add rounds to integer, RNE)
    p_i = (r_i - MAGIC) * k_i      (subtract is exact, result = round(u/scale)*k)
    out = p0 + p1 + p2 + mn        (written to `out`, any dtype)
    If delta_out given: also delta_out[0:delta_rows] =
        (acc + mn) - delta_src   on the first delta_rows partitions.
    Returns the output tile ([nparts, nfree]).
    """
    eng = eng if eng is not None else nc.vector
    if u_pre is not None:
        u = u_pre
    else:
        u = pool.tile([nparts, nfree], F32, tag=f"qu_{nparts}_{nfree}{sfx}", name="qu")
        eng.tensor_scalar(u, src, cbuf[:, 6:7], None, op0=ALU.subtract)
    p = []
    for i in range(3):
        # all on DVE/Pool, in place: per-op IEEE fp32 rounding must match the
        # reference's separate mul/add ops (ACT's fused internal arithmetic
        # flips near-tie elements into the next quant bucket on HW)
        pi = pool.tile(
            [nparts, nfree], F32, tag=f"ptmp{i}_{nparts}_{nfree}{sfx}", name=f"ptmp{i}"
        )
        eng.tensor_scalar(pi, u, cbuf[:, i : i + 1], None, op0=ALU.mult)
        eng.tensor_scalar(pi, pi, MAGIC, None, op0=ALU.add)
        eng.tensor_scalar(
            pi, pi, MAGIC, cbuf[:, 3 + i : 4 + i], op0=ALU.subtract, op1=ALU.mult
        )
        p.append(pi)
    eng.tensor_add(p[0], p[0], p[1])
    eng.tensor_add(p[0], p[0], p[2])
    outt = out if out is not None else pool.tile(
        [nparts, nfree], F32, tag=f"qout_{nparts}_{nfree}{sfx}", name="qout"
    )
    eng.tensor_scalar(outt, p[0], cbuf[:, 6:7], None, op0=ALU.add)
    if delta_out is not None:
        nc.vector.scalar_tensor_tensor(
            delta_out[0:delta_rows, :],
            p[0][0:delta_rows, :],
            cbuf[0:delta_rows, 6:7],
            delta_src[0:delta_rows, :],
            op0=ALU.add,
            op1=ALU.subtract,
        )
    return outt


def _kernel_body(ctx, tc, ch, x_ap, selred_ap, selloc_ap, w_ap, ws_ap, al_ap, out_ap):
    nc = tc.nc
    import concourse.bass_isa as bass_isa

    const = ctx.enter_context(tc.tile_pool(name="const", bufs=1))
    rhs_pool = ctx.enter_context(tc.tile_pool(name="rhs", bufs=4))
    out_pool = ctx.enter_context(tc.tile_pool(name="outsb", bufs=2))
    psB = ctx.enter_context(tc.tile_pool(name="psB", bufs=8, space="PSUM"))

    # ---- inputs ----
    alphas = const.tile([1, 6], F32)
    nc.gpsimd.dma_start(alphas[:], al_ap)
    # W arrives pre-transposed and channel-permuted from the host
    wtside = const.tile([128, 2 * C], F32)  # W^T chunks side by side
    nc.sync.dma_start(wtside[:, 0:C], w_ap[0:128, :])
    nc.sync.dma_start(wtside[:, C : 2 * C], w_ap[128:256, :])
    wseltraw = const.tile([NSEL, C], F32)
    nc.sync.dma_start(wseltraw[:], ws_ap)
    # replicated gathered-channel copy for the global min/max.  A DMA
    # occupies its issuing queue for the whole transfer, so spread: c0/c1
    # on ACT (idle until evictions), c2 on SP before the batch-0 rhs
    # stream, c3 on SP right after it.  Pool stays clear for the W path
    # (which gates the first matmul).
    selredc = [
        const.tile([128, 2048], F32, name=f"selredc{i}", tag=f"selredc{i}")
        for i in range(4)
    ]
    nc.sync.dma_start(selredc[2][:], selred_ap[:, 2 * 2048 : 3 * 2048])
    nc.sync.dma_start(selredc[3][:], selred_ap[:, 3 * 2048 : 4 * 2048])
    nc.scalar.dma_start(selredc[0][:], selred_ap[:, 0:2048])
    nc.scalar.dma_start(selredc[1][:], selred_ap[:, 2048 : 2 * 2048])
    selloc = const.tile([128, 1024], F32)
    nc.scalar.dma_start(selloc[:], selloc_ap)
    # prefetch all four batches' rhs tiles (the SP queue streams them
    # back-to-back; the scatters ride SP afterwards so they can never
    # head-of-line-block a load)
    rhs_t = []
    for b in range(BPC):
        pair = []
        for k in range(2):
            t = rhs_pool.tile([128, HW], F16, tag=f"rhs{k}")
            nc.sync.dma_start(t[:], x_ap[b, k * 128 : (k + 1) * 128, :])
            pair.append(t)
        rhs_t.append(pair)

    with tc.high_priority():
        # ---- softmax of both alpha vectors (on partition 0) ----
        ex = const.tile([1, 6], F32)
        nc.scalar.activation(ex[:], alphas[:], ACTF.Exp)
        sums = const.tile([1, 8], F32)
        nc.vector.tensor_reduce(sums[0:1, 0:1], ex[0:1, 0:3], axis=AXIS.X, op=ALU.add)
        nc.vector.tensor_reduce(sums[0:1, 1:2], ex[0:1, 3:6], axis=AXIS.X, op=ALU.add)
        nc.vector.reciprocal(sums[0:1, 2:3], sums[0:1, 0:1])
        nc.vector.reciprocal(sums[0:1, 3:4], sums[0:1, 1:2])
        sw = const.tile([1, 6], F32)  # cols 0..2 = sw_activ, 3..5 = sw_weight
        nc.vector.tensor_scalar(sw[0:1, 0:3], ex[0:1, 0:3], sums[0:1, 2:3], None, op0=ALU.mult)
        sw_last = nc.vector.tensor_scalar(
            sw[0:1, 3:6], ex[0:1, 3:6], sums[0:1, 3:4], None, op0=ALU.mult
        )

        # qmax and fl(1/qmax) constant vectors for the exact-division sequence
        d3 = const.tile([1, 3], F32)
        y3 = const.tile([1, 3], F32)
        for i, qm in enumerate(QMAX):
            nc.gpsimd.memset(d3[0:1, i : i + 1], float(qm))
            nc.gpsimd.memset(y3[0:1, i : i + 1], float(np.float32(1.0) / np.float32(qm)))

        # ---- W min/max: DVE free-axis partials + gpsimd cross-partition ----
        scal = const.tile([1, 8], F32)  # 0 smx, 1 smn, 2 wmx, 3 wmn
        wp = const.tile([128, 2], F32)
        wg = const.tile([128, 2], F32)
        nc.vector.tensor_reduce(wp[:, 0:1], wtside[:], axis=AXIS.X, op=ALU.max)
        nc.vector.tensor_reduce(
            wp[:, 1:2], wtside[:], axis=AXIS.X, op=ALU.min, negate=True
        )
        nc.gpsimd.partition_all_reduce(
            wg[:, 0:2], wp[:, 0:2], channels=128, reduce_op=bass_isa.ReduceOp.max
        )
        nc.gpsimd.tensor_copy(scal[0:1, 2:3], wg[0:1, 0:1])
        nc.gpsimd.tensor_scalar(scal[0:1, 3:4], wg[0:1, 1:2], -1.0, None, op0=ALU.mult)
        mnbw = const.tile([128, 1], F32)
        nc.gpsimd.partition_broadcast(mnbw[:], scal[0:1, 3:4])
        uw = const.tile([128, 2 * C], F32)
        nc.gpsimd.tensor_scalar(uw[:], wtside[:], mnbw[:, 0:1], None, op0=ALU.subtract)

        # ---- W consts + quantized weights (fp16 lhsT) ----
        valsw = const.tile([1, 10], F32)
        tmpw = const.tile([1, 40], F32)
        recip_w = _emit_scalar_consts(
            nc, valsw, scal[0:1, 2:3], scal[0:1, 3:4], sw[0:1, 3:6], tmpw, d3, y3,
            eng=nc.gpsimd,
        )
        cbufw = const.tile([128, 10], F32)
        nc.gpsimd.partition_broadcast(cbufw[:], valsw[0:1, :])
        lhsT = [
            const.tile([128, C], F16, name=f"lhsT{k}", tag=f"lhsT{k}") for k in range(2)
        ]
        # m0 column halves of both k-chunks first: the first main-matmul
        # group reads only lhsT[k][:, 0:128]
        _emit_quant(
            nc, const, wtside[:, 0:128], cbufw, 128, 128,
            out=lhsT[0][:, 0:128], eng=nc.gpsimd, sfx="w0a", u_pre=uw[:, 0:128],
        )
        _emit_quant(
            nc, const, wtside[:, C : C + 128], cbufw, 128, 128,
            out=lhsT[1][:, 0:128], eng=nc.gpsimd, sfx="w1a", u_pre=uw[:, C : C + 128],
        )

    # Pool takes chunk 3's stats entirely via its (slow but parallel)
    # all-axis reduce, emitted (priority-wise) between the m0 and m1
    # weight quants; min = -max(-x) since the cross-lane path has no min
    c3s = const.tile([1, 2], F32)  # 0: max(c3), 1: max(-c3)
    sneg = const.tile([128, 2048], F32)
    nc.gpsimd.tensor_reduce(c3s[0:1, 0:1], selredc[3][:], axis=AXIS.XYZWC, op=ALU.max)
    nc.gpsimd.tensor_scalar(sneg[:], selredc[3][:], -1.0, None, op0=ALU.mult)
    nc.gpsimd.tensor_reduce(c3s[0:1, 1:2], sneg[:], axis=AXIS.XYZWC, op=ALU.max)

    if True:
        _emit_quant(
            nc, const, wtside[:, 128:256], cbufw, 128, 128,
            out=lhsT[0][:, 128:256], eng=nc.gpsimd, sfx="w0b", u_pre=uw[:, 128:256],
        )
        _emit_quant(
            nc, const, wtside[:, C + 128 : 2 * C], cbufw, 128, 128,
            out=lhsT[1][:, 128:256], eng=nc.gpsimd, sfx="w1b", u_pre=uw[:, C + 128 : 2 * C],
        )
        # correction weights (batch 0 only): quantize W[:, ch]^T directly.
        # fp16(quant) here equals the lhsT rows for those channels exactly.
        corrT = const.tile([NSEL, C], F16)
        _emit_quant(
            nc, const, wseltraw[:], cbufw[0:NSEL, :], NSEL, C,
            out=corrT[:], eng=nc.gpsimd, sfx="ws",
        )
        # K=64 zero-padded correction weights, batch-0 variants only:
        # corrT64[q] holds corrT rows at partition offset q*8. Full-height
        # tiles so base_partition is 0 (PE 64-row tiles need bases {0,64}).
        corrT64 = [
            const.tile([128, C], F16, name=f"corrT64_{q}", tag=f"corrT64_{q}")
            for q in range(4)
        ]
        for q in range(4):
            nc.gpsimd.memset(corrT64[q][0:64, :], 0.0)
            nc.scalar.dma_start(corrT64[q][q * 8 : q * 8 + NSEL, :], corrT[:])

    # ---- sel min/max: fused elementwise+reduce (TTR) on DVE ----
    # selp cols: 0 max(c0,c1), 1 max(c2,c3), 2 -min(c0,c1), 3 -min(c2,c3)
    # per-chunk partials on DVE (TTR fused reduce crashes TRN2 HW here):
    # c0/c1 reduced directly, (c2,c3) via the Pool pair outputs. The
    # explicit deps force the tiny DVE ops of the weights path (softmax
    # tail, reciprocal) ahead of these long reductions in the static DVE
    # order — otherwise the scheduler's criticality heuristic starves the
    # W pipeline for ~14us.
    from concourse.tile import add_dep_helper

    selp = const.tile([128, 8], F32)
    rlist = []
    for i in (2, 0, 1):
        rlist.append(nc.vector.tensor_reduce(
            selp[:, i : i + 1], selredc[i][:], axis=AXIS.X, op=ALU.max
        ))
        rlist.append(nc.vector.tensor_reduce(
            selp[:, 4 + i : 5 + i], selredc[i][:], axis=AXIS.X, op=ALU.min,
            negate=True,
        ))
    for r in rlist:
        add_dep_helper(r.ins, recip_w.ins, reason="W consts before sel reductions")
        add_dep_helper(r.ins, sw_last.ins, reason="softmax tail before sel reductions")
    sc2 = const.tile([128, 2], F32)
    nc.vector.tensor_reduce(sc2[:, 0:1], selp[:, 0:3], axis=AXIS.X, op=ALU.max)
    nc.vector.tensor_reduce(sc2[:, 1:2], selp[:, 4:7], axis=AXIS.X, op=ALU.max)
    gred = const.tile([128, 2], F32)
    nc.gpsimd.partition_all_reduce(
        gred[:, 0:2], sc2[:, 0:2], channels=128, reduce_op=bass_isa.ReduceOp.max
    )
    # fold in Pool's chunk-3 scalars: scal[0]=max, scal[1]=min
    nc.vector.tensor_scalar(
        scal[0:1, 0:1], gred[0:1, 0:1], c3s[0:1, 0:1], None, op0=ALU.max
    )
    nc.vector.tensor_scalar(
        scal[0:1, 4:5], gred[0:1, 1:2], c3s[0:1, 1:2], None, op0=ALU.max
    )
    nc.vector.tensor_scalar(scal[0:1, 1:2], scal[0:1, 4:5], -1.0, None, op0=ALU.mult)

    # ---- sel consts + activ (fp16) + batch-0 delta ----
    valss = const.tile([1, 10], F32)
    tmps = const.tile([1, 40], F32)
    _emit_scalar_consts(
        nc, valss, scal[0:1, 0:1], scal[0:1, 1:2], sw[0:1, 0:3], tmps, d3, y3
    )
    cbufs = const.tile([128, 10], F32)
    nc.gpsimd.partition_broadcast(cbufs[:], valss[0:1, :])
    activ = const.tile([128, 1024], F16)
    delta0 = const.tile([128, 1024], F16)
    nc.gpsimd.memset(delta0[32:64, :], 0.0)
    # column-split across DVE and gpsimd (576/448 balances their measured
    # per-op rates); batch-0 delta rows (0..31) come from the same fp32
    # accumulator so the correction is fp16-consistent
    SPL = 576
    _emit_quant(
        nc, const, selloc[:, 0:SPL], cbufs, 128, SPL,
        out=activ[:, 0:SPL], eng=nc.vector, sfx="sa",
        delta_out=delta0[:, 0:SPL], delta_src=selloc[:, 0:SPL], delta_rows=32,
    )
    _emit_quant(
        nc, const, selloc[:, SPL:1024], cbufs, 128, 1024 - SPL,
        out=activ[:, SPL:1024], eng=nc.gpsimd, sfx="sb",
        delta_out=delta0[:, SPL:1024], delta_src=selloc[:, SPL:1024], delta_rows=32,
    )

    # ---- main loop ----
    for b in range(BPC):
        rhs0, rhs1 = rhs_t[b]
        if b > 0:
            # patch the 8 sel rows with the quantized activations (on SP,
            # behind every rhs load, so they can't block a load)
            for q in range(4):
                p0 = b * 32 + q * 8
                nc.sync.dma_start(
                    rhs1[SELP0 : SELP0 + NSEL, q * 1024 : (q + 1) * 1024],
                    activ[p0 : p0 + NSEL, :],
                )
        for m in range(2):
            outsb = out_pool.tile([128, HW], F16, name="outsb", tag="outsb")
            for g in range(4):  # two [128,512] PSUM tiles (1 bank each) per group
                gi = ((b * 2 + m) * 4 + g)
                pts = []
                for h in range(2):
                    n = g * 2 + h
                    pt = psB.tile([128, 512], F32, name="ptile", tag="ptile")
                    pts.append(pt)
                    nc.tensor.matmul(
                        pt[:],
                        lhsT[0][:, m * 128 : (m + 1) * 128],
                        rhs0[:, n * 512 : (n + 1) * 512],
                        start=True,
                        stop=False,
                    )
                    nc.tensor.matmul(
                        pt[:],
                        lhsT[1][:, m * 128 : (m + 1) * 128],
                        rhs1[:, n * 512 : (n + 1) * 512],
                        start=False,
                        stop=(b > 0),
                    )
                    if b == 0:
                        q, r = divmod(n, 2)
                        nc.tensor.matmul(
                            pt[:],
                            corrT64[q][0:64, m * 128 : (m + 1) * 128],
                            delta0[0:64, r * 512 : (r + 1) * 512],
                            start=False,
                            stop=True,
                        )
                # eviction with fp32 -> fp16 cast: DVE is ~2x faster than
                # ACT at this, so it takes 5 of 8 per (b, m)
                for h in range(2):
                    n = g * 2 + h
                    if (g * 2 + h) in (1, 3, 6):
                        nc.scalar.copy(outsb[:, n * 512 : (n + 1) * 512], pts[h][:])
                    else:
                        nc.vector.tensor_copy(
                            outsb[:, n * 512 : (n + 1) * 512], pts[h][:]
                        )
            is_last = b == BPC - 1 and m == 1
            if is_last:
                # final drain per eviction so the tail stays short
                for g in range(4):
                    eng = (nc.scalar, nc.gpsimd, nc.sync, nc.gpsimd)[g % 4]
                    eng.dma_start(
                        out_ap[b, m * 128 : (m + 1) * 128, g * 1024 : (g + 1) * 1024],
                        outsb[:, g * 1024 : (g + 1) * 1024],
                    )
            else:
                # one big out-DMA per (b, m), mostly on the otherwise-idle
                # Pool queue (SP takes two late ones)
                eng = (nc.gpsimd, nc.gpsimd, nc.gpsimd, nc.gpsimd,
                       nc.sync, nc.gpsimd, nc.sync)[b * 2 + m]
                eng.dma_start(out_ap[b, m * 128 : (m + 1) * 128, :], outsb[:])


def build_program(ch):
    nc = bacc.Bacc(
        "TRN2", target_bir_lowering=False, debug=False, num_devices=NCORES
    )
    x_t = nc.dram_tensor("x", [BPC, C, HW], F16, kind="ExternalInput").ap()
    selred_t = nc.dram_tensor("selred", [128, 8192], F32, kind="ExternalInput").ap()
    selloc_t = nc.dram_tensor("selloc", [128, 1024], F32, kind="ExternalInput").ap()
    w_t = nc.dram_tensor("wt", [C, C], F32, kind="ExternalInput").ap()
    ws_t = nc.dram_tensor("wselt", [NSEL, C], F32, kind="ExternalInput").ap()
    al_t = nc.dram_tensor("alphas", [1, 6], F32, kind="ExternalInput").ap()
    out_t = nc.dram_tensor("out", [BPC, C, HW], F16, kind="ExternalOutput").ap()
    with tile.TileContext(nc) as tc:
        with ExitStack() as ctx:
            _kernel_body(
                ctx, tc, ch, x_t, selred_t, selloc_t, w_t, ws_t, al_t, out_t
            )
    nc.compile()
    return nc


def make_in_maps(x, alpha_activ, alpha_weight, conv_weight, selected_channels):
    x = np.ascontiguousarray(np.asarray(x, dtype=np.float32).reshape(B, C, HW))
    ch = [int(v) for v in np.asarray(selected_channels).ravel()]
    sel = np.ascontiguousarray(x[:, ch, :])  # [32, 8, 4096]
    selred = sel.reshape(128, 8192)
    alphas = np.concatenate(
        [np.asarray(alpha_activ).ravel(), np.asarray(alpha_weight).ravel()]
    ).astype(np.float32).reshape(1, 6)
    wmat = np.asarray(conv_weight, dtype=np.float32).reshape(C, C)
    # channel permutation: all sel channels at positions 248..255 (k-chunk-1
    # partitions 120..127); wt rows permuted to match (the contraction is
    # permutation-invariant)
    nonsel = [c for c in range(C) if c not in set(ch)]
    perm = nonsel + ch  # position i holds original channel perm[i]
    xp = x[:, perm, :]
    wt = np.ascontiguousarray(wmat.T[perm, :])
    wselt = np.ascontiguousarray(wmat[:, ch].T)  # [8, 256]
    in_maps = []
    for c in range(NCORES):
        xs = np.ascontiguousarray(xp[c * BPC : (c + 1) * BPC].astype(np.float16))
        # selloc layout: partition p = b*32 + q*8 + j holds
        # sel[core*4+b, j, q*1024 : (q+1)*1024]
        sl = sel[c * BPC : (c + 1) * BPC].reshape(BPC, NSEL, 4, 1024)
        selloc = np.ascontiguousarray(sl.transpose(0, 2, 1, 3).reshape(128, 1024))
        in_maps.append(
            {
                "x": xs,
                "selred": selred,
                "selloc": selloc,
                "wt": wt,
                "wselt": wselt,
                "alphas": alphas,
            }
        )
    return ch, in_maps


def kernel(x, alpha_activ, alpha_weight, conv_weight, selected_channels):
    from concourse.bass_utils import run_bass_kernel_spmd

    ch, in_maps = make_in_maps(
        x, alpha_activ, alpha_weight, conv_weight, selected_channels
    )
    nc = build_program(ch)
    res = run_bass_kernel_spmd(nc, in_maps, core_ids=list(range(NCORES)))
    outs = [
        res.results[c]["out"].astype(np.float32).reshape(BPC, C, H, W)
        for c in range(NCORES)
    ]
    return np.concatenate(outs, axis=0)
